# revision 3
# baseline (speedup 1.0000x reference)
"""CLVP attention kernel for 8 Trainium2 NeuronCores — v3 (W-stationary projections).

Sharding: core c = 2*b + hg handles batch b and head-group hg (8 of 16 heads),
processed as 4 pair-groups (2 heads = 128 dims each).

v3 vs v2 structural changes:
  - Q/K projections run W-stationary (lhsT = W tile [emb,dim], rhs = hid
    [emb,tok], N=512), producing qT/kT in [dim, tok] layout DIRECTLY — no PE
    transposes, no transpose drains, 256 instead of 1024 projection matmuls
    (real-HW LDWEIGHTS is no longer the bottleneck; each 107ns load hides
    under a 213ns matmul).
  - Rotary on q/k is applied in the transposed layout via the "u-trick":
      u   = rsin2 (.) proj_psum      (per-source-row signed sin, DVE)
      qT  = rcos  (.) proj_psum      (cos factors; rows >= 32 are ones, DVE)
      sh  = Pm^T @ u                 (partition permutation as a PE matmul)
      qT += sh                       (DVE)
    where rsin2[d] = s_{sigma(d)} * sin_{sigma(d)} and Pm[sigma(j), j] = 1.
    Validated exactly against the reference rotate-half formulation.
  - V projection is hid-stationary with N=512 moving covering ALL 4 groups'
    head dims at once (128 matmuls), rotary applied per token-tile from PSUM
    into one big vext tile [128, (g kc h 65)] whose 65th column is ones
    (softmax denominator rides the AV matmul for free).
  - Startup is software-pipelined: attention for group 0 starts after only
    kT chunk 0..3 / qT chunk 0 / vext tiles 0..3 exist; remaining V tiles and
    q chunks interleave into the first attention qc-blocks, so the ScalarE
    exp stream (the ~266us/core floor) starts ~13us in.
  - Projections for group g+1 interleave into attention(g) qc-blocks in two
    phases (proj+muls first, Pm+add one block later) so the Pm matmul never
    stalls the in-order PE stream waiting on DVE.

Unchanged from v2 (do not re-break): attention block (scores transposed,
tile_position-packed QK pairs, exp on ScalarE, fused V|ones AV matmuls,
1/z re-broadcast via K=1 matmul into just-freed PSUM banks), deferred
out-projection in group 3. PSUM = exactly 8 banks (4 sps + outA + outB +
2 scratch shared by proj/P-shift/V-proj/out-proj).

Known-bad variants (v2 postmortem — do not re-add): gpsimd
partition_broadcast (HW race), fp8e4 DoubleRow matmuls interleaved with
bf16 (NRT_EXEC_UNIT_UNRECOVERABLE device fault).
"""

import numpy as np
import ml_dtypes

import concourse.bass as bass
import concourse.tile as tile
from concourse import bacc, mybir
from concourse.bass_utils import run_bass_kernel_spmd

B, S, E, H, D, ROT = 4, 2048, 1024, 16, 64, 32
HLOC = 8            # heads per core
HS = HLOC * D       # 512 head dims per core
G = 4               # pair-groups per core (2 heads each)
N_CORES = 8
KE = E // 128       # 8 contraction tiles for projections
TT = S // 128       # 16 token tiles
QC = S // 512       # 4 q chunks
KC = S // 128       # 16 k chunks
NCH = S // 512      # 4 projection chunks of 512 tokens

f32 = mybir.dt.float32
f32r = mybir.dt.float32r
bf16 = mybir.dt.bfloat16
FT = mybir.ActivationFunctionType


def _emit(nc, tc, ctx, t):
    (hidT, wq, wk, wv, m1, aux, cs, part) = t
    w_dram = {"q": wq, "k": wk, "v": wv}

    const = ctx.enter_context(tc.tile_pool(name="const", bufs=1))
    # ones built on-device (no DMA): zps-broadcast column + vext denominator.
    # Both memsets go FIRST on the Pool queue — behind the Pool DMAs they
    # would gate the first AV accumulation until ~20us in.
    ones_f = const.tile([128, 64], f32r)
    nc.gpsimd.memset(ones_f[:].bitcast(f32), 1.0)
    vext = const.tile([128, G * KC * 2 * 65], bf16, name="vext")
    vext_v = vext[:].rearrange("p (g kc h c) -> p g kc h c", kc=KC, h=2, c=65)
    nc.gpsimd.memset(vext_v[:, :, :, :, 64:65], 1.0)
    # DMAs are spread over three engine queues (SP/ACT/Pool). hidT arrives
    # host-repacked token-chunk-major ([c][E][512]) so ONE 1MB DMA delivers
    # everything the first 512-token projection chunk needs (contiguous 8KB
    # per partition); chunks 1-3 stream in behind the attention consumption
    # order (chunk c is first needed at kc=4c).
    hid_sb = const.tile([128, NCH * KE * 512], bf16)
    hid_v = hid_sb[:].rearrange("p (c k s) -> p c k s", k=KE, s=512)
    hid_ap = hidT.ap().rearrange("(c k p) s -> p c k s", c=NCH, p=128)
    w_sb = {}
    for name in ("k", "q", "v"):
        w_sb[name] = const.tile([128, KE * HS], bf16, name=f"w{name}", tag=f"w{name}")
    aux_sb = const.tile([128, 128 + 2 * S], bf16, name="aux")
    pm_sb = aux_sb[:, 0:128]
    rcos_sb = aux_sb[:, 128 : 128 + S]
    rsin2_sb = aux_sb[:, 128 + S : 128 + 2 * S]
    csb = const.tile([128, TT * 96], bf16, name="csb")
    m1_sb = const.tile([128, G * E], bf16)

    def _wdma(eng, name):
        eng.dma_start(
            w_sb[name][:].rearrange("p (k n) -> p k n", n=HS),
            w_dram[name].ap().rearrange("(k p) n -> p k n", p=128),
        )

    def _hdma(eng, c):
        eng.dma_start(hid_v[:, c : c + 1], hid_ap[:, c : c + 1])

    # The sim (and HW descriptor rings) serialize transfers roughly in
    # issue order across the three queues, so the queue heads carry the
    # critical chain (wk+c0 -> k-proj, wq -> q-proj, aux -> rotary muls)
    # and bulk tokens/weights needed later ride behind.
    # SP queue
    _hdma(nc.sync, 0)
    _wdma(nc.sync, "q")
    _hdma(nc.sync, 1)
    _hdma(nc.sync, 3)
    _hdma(nc.sync, 2)
    # ACT queue (kept short: a queued DMA holds ACT.SEQ through its
    # transfer and would gate the first exp dispatches)
    _wdma(nc.scalar, "k")
    nc.scalar.dma_start(aux_sb[:], aux.ap())
    # Pool queue
    _wdma(nc.gpsimd, "v")
    nc.gpsimd.dma_start(
        csb[:].rearrange("p (t d) -> p t d", d=96),
        cs.ap().rearrange("(t p) d -> p t d", p=128),
    )
    nc.gpsimd.dma_start(
        m1_sb[:].rearrange("p (t n) -> p t n", n=E),
        m1.ap().rearrange("(t p) n -> p t n", p=128),
    )

    qk_pool = ctx.enter_context(tc.tile_pool(name="qk", bufs=8))
    u_pool = ctx.enter_context(tc.tile_pool(name="u", bufs=6))
    tmp_pool = ctx.enter_context(tc.tile_pool(name="tmp", bufs=4))
    ex_pool = ctx.enter_context(tc.tile_pool(name="exp", bufs=8))
    onorm_pool = ctx.enter_context(tc.tile_pool(name="onorm", bufs=16))
    oas_pool = ctx.enter_context(tc.tile_pool(name="oas", bufs=2))
    rz_pool = ctx.enter_context(tc.tile_pool(name="rz", bufs=2))
    ws_pool = ctx.enter_context(tc.tile_pool(name="ws", bufs=2))
    psum = ctx.enter_context(tc.tile_pool(name="psum", bufs=1, space="PSUM"))

    # per-(g, X) chunk tiles of qT/kT: [128 dims, 512 tok] each
    qk_tiles = {}
    # phase-A leftovers: (g, X, c) -> u tile
    pend = {}
    onorm_tiles = {}
    # global filler queue: (tag, fn) units of <=1us PE work, popped between
    # attention kc iterations so projection/V/out-proj work fills the PE
    # slack inside the ACT-paced inner loop instead of stalling it.
    filler_q = []

    def pop_fillers(n):
        for _ in range(n):
            if not filler_q:
                return
            filler_q.pop(0)[1]()

    def flush_fillers(tag):
        while any(t == tag for t, _ in filler_q):
            filler_q.pop(0)[1]()

    def _proj_mm(psA, g, X, c, k):
        nc.tensor.matmul(
            psA[:],
            w_sb[X][:, HS * k + 128 * g : HS * k + 128 * (g + 1)],
            hid_v[:, c, k, :],
            start=(k == 0),
            stop=(k == KE - 1),
        )

    def _proj_muls(psA, g, X, c):
        u = u_pool.tile([128, 512], bf16, tag="u", name="u")
        nc.vector.tensor_mul(u[:], psA[:], rsin2_sb[:, 512 * c : 512 * (c + 1)])
        dest = qk_pool.tile([128, 512], bf16, tag=f"{X}T", name=f"{X}T")
        qk_tiles[(g, X, c)] = dest
        nc.vector.tensor_mul(dest[:], psA[:], rcos_sb[:, 512 * c : 512 * (c + 1)])
        pend[(g, X, c)] = u

    def proj_a(g, X, c):
        """Projection chunk phase A: 8 accumulating matmuls + u/cos muls."""
        psA = psum.tile([128, 512], f32, tag="scr", bufs=2, name="psA")
        for k in range(KE):
            _proj_mm(psA, g, X, c, k)
        _proj_muls(psA, g, X, c)

    def proj_b(g, X, c):
        """Projection chunk phase B: Pm shift matmul + add."""
        u = pend.pop((g, X, c))
        dest = qk_tiles[(g, X, c)]
        sh = psum.tile([128, 512], f32, tag="scr", bufs=2, name="sh")
        nc.tensor.matmul(sh[:], pm_sb[:], u[:], start=True, stop=True)
        nc.vector.tensor_add(dest[:], dest[:], sh[:])

    def proj_stream_units(tag, g, chunks):
        """Unit-ified projections for group g: per chunk 4 A-units of 2
        matmuls (last also does the u/cos muls) and 1 B-unit (Pm + add),
        B(c) ordered after A(c+1) for DVE lead time."""
        state = {}

        def a_unit(X, c, k0):
            def f():
                if k0 == 0:
                    state[(X, c)] = psum.tile(
                        [128, 512], f32, tag="scr", bufs=2, name="psA"
                    )
                psA = state[(X, c)]
                _proj_mm(psA, g, X, c, k0)
                _proj_mm(psA, g, X, c, k0 + 1)
                if k0 == KE - 2:
                    _proj_muls(psA, g, X, c)
            return f

        units = []
        prev = None
        for X, c in chunks:
            units += [(tag, a_unit(X, c, k0)) for k0 in (0, 2, 4, 6)]
            if prev is not None:
                Xp, cp = prev
                units.append((tag, lambda Xp=Xp, cp=cp: proj_b(g, Xp, cp)))
            prev = (X, c)
        Xp, cp = prev
        units.append((tag, lambda Xp=Xp, cp=cp: proj_b(g, Xp, cp)))
        return units

    def _vtile_mm(ppv, tt, k):
        nc.tensor.matmul(
            ppv[:],
            hid_v[:, tt // 4, k, 128 * (tt % 4) : 128 * (tt % 4) + 128],
            w_sb["v"][:, HS * k : HS * (k + 1)],
            start=(k == 0),
            stop=(k == KE - 1),
        )

    def vtile(tt):
        """V projection + rotary for token tile tt (all 4 groups at once)."""
        ppv = psum.tile([128, 512], f32, tag="scr", bufs=2, name="ppv")
        for k in range(KE):
            _vtile_mm(ppv, tt, k)
        _vtile_rot(ppv, tt)

    def _vtile_rot(ppv, tt):
        psv = ppv[:].rearrange("p (g h d) -> p g h d", h=2, d=64)
        dst = vext_v[:, :, tt, :, 0:64]
        cb = (
            csb[:, 96 * tt : 96 * tt + 64]
            .rearrange("p (a b d) -> p a b d", a=1, b=1)
            .broadcast_to([128, G, 2, 64])
        )
        nc.vector.tensor_mul(dst, psv, cb)
        tmp = tmp_pool.tile([128, G * 2 * 32], bf16, tag="tmp")
        tmpv = tmp[:].rearrange("p (g h d) -> p g h d", h=2, d=32)
        sv = csb[:, 96 * tt + 64 : 96 * tt + 96].rearrange(
            "p (a b d) -> p a b d", a=1, b=1
        )
        s0 = sv[:, :, :, 0:16].broadcast_to([128, G, 2, 16])
        s1 = sv[:, :, :, 16:32].broadcast_to([128, G, 2, 16])
        nc.vector.tensor_mul(tmpv[:, :, :, 0:16], psv[:, :, :, 16:32], s0)
        nc.vector.tensor_mul(tmpv[:, :, :, 16:32], psv[:, :, :, 0:16], s1)
        rotslice = dst[:, :, :, 0:32]
        nc.vector.tensor_add(rotslice, rotslice, tmpv)

    def vtile_units(tt):
        """vtile as two filler units of 4 matmuls (rotary on the second)."""
        state = {}

        def u1():
            state["ppv"] = psum.tile([128, 512], f32, tag="scr", bufs=2, name="ppv")
            for k in range(4):
                _vtile_mm(state["ppv"], tt, k)

        def u2():
            for k in range(4, KE):
                _vtile_mm(state["ppv"], tt, k)
            _vtile_rot(state["ppv"], tt)

        return [("v", u1), ("v", u2)]

    def outproj_units(qc):
        """Out-projection for q-chunk qc as 16 filler units of 2 matmuls;
        all 8 m-slices collect into one SBUF buffer -> single output DMA."""
        state = {}

        def mk(m, p0):
            def f():
                if m == 0 and p0 == 0:
                    state["ws"] = ws_pool.tile(
                        [128, (E // 128) * 512], bf16, tag="ws", name="ws"
                    )
                if p0 == 0:
                    state["wps"] = psum.tile(
                        [128, 512], f32, tag="scr", bufs=2, name="wps"
                    )
                wps = state["wps"]
                for p in (p0, p0 + 1):
                    nc.tensor.matmul(
                        wps[:],
                        m1_sb[:, E * p + 128 * m : E * p + 128 * (m + 1)],
                        onorm_tiles[(p, qc)][:],
                        start=(p == 0),
                        stop=(p == G - 1),
                    )
                if p0 == 2:
                    ws_v = state["ws"][:].rearrange("p (m s) -> p m s", s=512)
                    nc.vector.tensor_copy(ws_v[:, m, :], wps[:])
                    # two half DMAs so the first half's writeback overlaps
                    # the second half's matmuls (matters for the last qc)
                    if m == E // 256 - 1 or m == E // 128 - 1:
                        mlo = 0 if m == E // 256 - 1 else E // 256
                        nc.gpsimd.dma_start(
                            part.ap()[128 * mlo : 128 * (m + 1),
                                      512 * qc : 512 * (qc + 1)].rearrange(
                                "(m p) s -> p m s", p=128
                            ),
                            ws_v[:, mlo : m + 1, :],
                        )
            return f

        return [(f"op{qc}", mk(m, p0)) for m in range(E // 128) for p0 in (0, 2)]

    AVLAG = 6

    def attn_block(g, qc, pops_per_kc, prev_tail=None):
        """One attention qc-block with lag-4 AV software pipelining: AV(kc)
        is emitted after QK(kc+4), so the in-order PE stream never parks on
        exp(kc) and the PREVIOUS block's softmax tail (emitted at kc==1)
        drains before this block's AV(0) needs the outA/outB banks. Filler
        units absorb the PE slack inside the ACT-paced loop. Returns this
        block's tail closure for the next block to emit."""
        qTs = qk_tiles[(g, "q", qc)]
        exs = {}
        # outA/outB allocated lazily at the first AV (kc==AVLAG): the
        # previous block's zps tiles (same tags, emitted at our kc==1)
        # must precede them in tag-rotation order.
        outs = []

        def av(kc):
            if not outs:
                outs.append(
                    psum.tile([65, 512], f32, tag="outA", bufs=1, name="outA")
                )
                outs.append(
                    psum.tile([65, 512], f32, tag="outB", bufs=1, name="outB")
                )
            outA, outB = outs
            ex = exs.pop(kc)
            nc.tensor.matmul(
                outA[:],
                vext_v[:, g, kc, 0, :],
                ex[:, 0:512],
                start=(kc == 0),
                stop=(kc == KC - 1),
            )
            nc.tensor.matmul(
                outB[:],
                vext_v[:, g, kc, 1, :],
                ex[:, 512:1024],
                start=(kc == 0),
                stop=(kc == KC - 1),
            )

        for kc in range(KC):
            kTs = qk_tiles[(g, "k", kc // 4)]
            ko = 128 * (kc % 4)
            sps = psum.tile([128, 1024], f32, tag="sps", bufs=2, name="sps")
            nc.tensor.matmul(
                sps[:, 0:512],
                kTs[0:64, ko : ko + 128],
                qTs[0:64, :],
                start=True,
                stop=True,
                tile_position=(0, 0),
            )
            nc.tensor.matmul(
                sps[:, 512:1024],
                kTs[64:128, ko : ko + 128],
                qTs[64:128, :],
                start=True,
                stop=True,
                tile_position=(64, 0),
            )
            ex = ex_pool.tile([128, 1024], bf16, tag="ex")
            nc.scalar.activation(ex[:], sps[:], FT.Exp)
            exs[kc] = ex
            if kc >= AVLAG:
                av(kc - AVLAG)
            if kc == 1 and prev_tail is not None:
                prev_tail()
            pop_fillers(pops_per_kc)
        for kc in range(KC - AVLAG, KC):
            av(kc)
        return lambda: attn_tail(g, qc, outs[0], outs[1])

    def attn_tail(g, qc, outA, outB):
        # softmax tail, restructured for a short PE-stall chain: reciprocal
        # reads the denominator row straight from PSUM, the drain copies only
        # rows 0:64, and the 1/z broadcast matmul lands in a scr bank (not
        # the outA/outB banks), so the next block's AV accumulation only
        # waits on the row drain.
        o_n = onorm_pool.tile([128, 512], bf16, tag="on", name="o_n")
        rzs, oXss = [], []
        for hh, outps in ((0, outA), (1, outB)):
            rz = rz_pool.tile([65, 512], f32r, tag="rz", name="rz")
            with nc.allow_low_precision(reason="softmax denom recip"):
                nc.vector.reciprocal(rz[64:65, :], outps[64:65, :])
            oXs = oas_pool.tile([64, 512], f32, tag=f"o{hh}", name="oXs")
            nc.vector.tensor_copy(oXs[:], outps[0:64, :])
            rzs.append(rz)
            oXss.append(oXs)
        for hh, otag in ((0, "outA"), (1, "outB")):
            zps = psum.tile([64, 512], f32, tag=otag, bufs=1, name="zps")
            nc.tensor.matmul(
                zps[:],
                ones_f[64:65, 0:64],
                rzs[hh][64:65, :],
                start=True,
                stop=True,
                tile_position=(64, 0),
            )
            nc.vector.tensor_mul(
                o_n[64 * hh : 64 * (hh + 1), :], oXss[hh][:], zps[:]
            )
        onorm_tiles[(g, qc)] = o_n
        if g == G - 1:
            filler_q.extend(outproj_units(qc))

    # ---------------- emission schedule ---------------------------------
    # Pre-phase (only what the qc=0 block needs up front): kT chunk 0,
    # qT chunk 0, V tiles 0-1. Everything else streams in as fillers.
    proj_a(0, "k", 0)
    proj_a(0, "q", 0)
    proj_b(0, "k", 0)
    proj_b(0, "q", 0)
    vtile(0)
    vtile(1)
    # Startup filler stream for block (0,0), interleaved to meet the
    # consumption deadlines at 3 pops/kc: k-chunk c is read from kc=4c,
    # vtile tt from the AV at loop index tt+2.
    sq = []
    sq += vtile_units(2) + vtile_units(3)
    sq += proj_stream_units("k1", 0, [("k", 1)])
    sq += vtile_units(4) + vtile_units(5)
    sq += proj_stream_units("k2", 0, [("k", 2)])
    sq += vtile_units(6) + vtile_units(7)
    sq += proj_stream_units("k3", 0, [("k", 3)])
    for tt in range(8, TT):
        sq += vtile_units(tt)
    filler_q.extend(sq)
    for c in (1, 2, 3):
        filler_q.extend(proj_stream_units(f"q{c}", 0, [("q", c)]))
    kq = [("k", 0), ("k", 1), ("k", 2), ("k", 3),
          ("q", 0), ("q", 1), ("q", 2), ("q", 3)]
    for g in (1, 2, 3):
        filler_q.extend(proj_stream_units(f"g{g}", g, kq))

    tail = None
    for g in range(G):
        for qc in range(QC):
            if (g, qc) == (0, 0):
                tail = attn_block(0, 0, pops_per_kc=3)
                continue
            if qc == 0 and g > 0:
                flush_fillers(f"g{g}")  # group g's own projections must be done
            elif g == 0:
                flush_fillers(f"q{qc}")  # this block's qT chunk
            tail = attn_block(
                g, qc, pops_per_kc=2 if g == 0 else 1, prev_tail=tail
            )
    tail()
    # drain: out-projection of the last q-chunk (+ anything left)
    while filler_q:
        filler_q.pop(0)[1]()


_NC_CACHE = {}


def _get_nc():
    if "nc" in _NC_CACHE:
        return _NC_CACHE["nc"]
    nc = bacc.Bacc("TRN2", target_bir_lowering=False, debug=False, num_devices=N_CORES)
    hidT = nc.dram_tensor("hidT", [NCH * E, 512], bf16, kind="ExternalInput")
    wq = nc.dram_tensor("wq", [E, HS], bf16, kind="ExternalInput")
    wk = nc.dram_tensor("wk", [E, HS], bf16, kind="ExternalInput")
    wv = nc.dram_tensor("wv", [E, HS], bf16, kind="ExternalInput")
    m1 = nc.dram_tensor("m1", [HS, E], bf16, kind="ExternalInput")
    aux = nc.dram_tensor("aux", [128, 128 + 2 * S], bf16, kind="ExternalInput")
    cs = nc.dram_tensor("cs", [S, 96], bf16, kind="ExternalInput")
    part = nc.dram_tensor("part", [E, S], bf16, kind="ExternalOutput")
    from contextlib import ExitStack

    with tile.TileContext(nc) as tc, ExitStack() as ctx:
        _emit(nc, tc, ctx, (hidT, wq, wk, wv, m1, aux, cs, part))
    nc.compile()
    _NC_CACHE["nc"] = nc
    return nc


def _in_maps(hidden_states, rotary_pos_emb, Wq, Wk, Wv, Wo):
    scale = np.float32(D**-0.5)
    f = np.asarray(rotary_pos_emb, np.float32)[0]  # [S, ROT]
    cs = np.ones((S, 96), np.float32)
    cs[:, 0:ROT] = np.cos(f)
    cs[:, 64:80] = -np.sin(f[:, 0:16])
    cs[:, 80:96] = np.sin(f[:, 16:ROT])
    cs = cs.astype(ml_dtypes.bfloat16)
    # aux = [Pm | rcos | rsin2] for q/k rotary in [dim, tok] layout
    aux = np.zeros((128, 128 + 2 * S), np.float32)
    for b in (0, 64):
        for j in range(16):
            aux[b + j + 16, b + j] = 1.0
            aux[b + j, b + j + 16] = 1.0
        aux[b : b + 32, 128 : 128 + S] = np.cos(f[:, 0:32]).T
        aux[b + 0 : b + 16, 128 + S :] = np.sin(f[:, 16:32]).T
        aux[b + 16 : b + 32, 128 + S :] = -np.sin(f[:, 0:16]).T
    aux[32:64, 128 : 128 + S] = 1.0
    aux[96:128, 128 : 128 + S] = 1.0
    aux = aux.astype(ml_dtypes.bfloat16)
    hs = np.asarray(hidden_states, np.float32)
    Wq, Wk, Wv, Wo = (np.asarray(w, np.float32) for w in (Wq, Wk, Wv, Wo))
    maps = []
    for c in range(N_CORES):
        b, hg = divmod(c, 2)
        rows = slice(hg * HS, (hg + 1) * HS)
        hid_t = hs[b].T  # [E, S]
        hid4 = np.ascontiguousarray(
            hid_t.reshape(E, NCH, 512).transpose(1, 0, 2).reshape(NCH * E, 512)
        )
        maps.append(
            {
                "hidT": hid4.astype(ml_dtypes.bfloat16),
                "wq": np.ascontiguousarray((Wq[rows] * scale).T).astype(ml_dtypes.bfloat16),
                "wk": np.ascontiguousarray(Wk[rows].T).astype(ml_dtypes.bfloat16),
                "wv": np.ascontiguousarray(Wv[rows].T).astype(ml_dtypes.bfloat16),
                "m1": np.ascontiguousarray(Wo[:, rows].T).astype(ml_dtypes.bfloat16),
                "aux": aux,
                "cs": cs,
            }
        )
    return maps


def kernel(hidden_states, rotary_pos_emb, Wq, Wk, Wv, Wo, bo, _trace=False):
    nc = _get_nc()
    maps = _in_maps(hidden_states, rotary_pos_emb, Wq, Wk, Wv, Wo)
    res = run_bass_kernel_spmd(
        nc, maps, core_ids=list(range(N_CORES)), trace=_trace
    )
    out = np.empty((B, S, E), np.float32)
    bo = np.asarray(bo, np.float32)
    for b in range(B):
        p0 = np.asarray(res.results[2 * b]["part"], dtype=np.float32)
        p1 = np.asarray(res.results[2 * b + 1]["part"], dtype=np.float32)
        out[b] = (p0 + p1).T + bo
    if _trace:
        kernel._last_results = res
    return out
